# revision 6
# baseline (speedup 1.0000x reference)
"""AreaAttention TRN2 kernel v3: fp16 QK + fp8-DoubleRow PV/denominator.

Math (per sample, C=128, N=4096):
    scores[m,n] = sum_c k[c,m] q[c,n];  k = x*colsum(Wk)+bk, q = Wq@x+bq
  bk adds a per-query constant to scores -> cancels in softmax. Folding
  wks=colsum(Wk) into Wq host-side (Wqw = wks[:,None]*Wq):
    scores_eff[m,n] = sum_c x[c,m] * qs[c,n],  qs = Wqw@x + bq*wks
  => no k tensor; the key-side QK operand is just x16.

Design notes (measured on HW):
  - fp16 matmul = 1 col/cycle @2.4GHz (216ns/512col). fp8 DoubleRow is only
    a real 2x when the contraction is genuinely 256: PV and the ones-rowsum
    contract over key pairs -> half the matmuls. The QK c-split [64,2,*]
    trick does NOT pay (64-partition shapes stream 1 col/cycle anyway), so
    QK stays fp16 (also better precision).
  - exp on ACT (exact exp -> fp8e5) + DVE (Schraudolph e5m2 u8 bits) at
    [128,512] grain through a 4-deep rotating PSUM sc pool.
  - denominator via ones8 DoubleRow rowsum on the PE (PSUM-accumulated per
    block) - no DVE chain adds at all.
  - GpSimd can't touch PSUM: it does SBUF residual adds + x16 DMA issue.
  - PSUM: pv[128,1024](2) + rs[128,1024](2) + 4x sc[128,512](4) = 8 banks.
  - PV/RS deferred one chunk so the in-order PE never stalls on fresh exps.
"""
import numpy as np
import ml_dtypes

C = 128
N = 4096
NB = 1024
NBLK = N // NB     # 4
MCH = N // C       # 32 m-chunks
NPAIR = MCH // 2   # 16
SCALE = 1.0 / np.sqrt(np.float32(C))
A5 = 4.0 / np.log(2.0)     # e5m2 Schraudolph slope
B5 = 59.75                 # e5m2 Schraudolph bias (HW cast rounds)

e4np = ml_dtypes.float8_e4m3

_cache = {}


def _build_nc():
    import concourse.tile as tile
    from concourse import bacc, mybir

    f32 = mybir.dt.float32
    f16 = mybir.dt.float16
    f8e4 = mybir.dt.float8e4
    f8e5 = mybir.dt.float8e5
    u8 = mybir.dt.uint8
    ADD = mybir.AluOpType.add
    MUL = mybir.AluOpType.mult
    EXP = mybir.ActivationFunctionType.Exp
    DR = mybir.MatmulPerfMode.DoubleRow

    nc = bacc.Bacc("TRN2", target_bir_lowering=False)

    x16_d = nc.dram_tensor("x16", [C, N], f16, kind="ExternalInput")
    # packed fp16 weights: [WqwT | WvT]
    w16_d = nc.dram_tensor("w16", [C, 2 * C], f16, kind="ExternalInput")
    smalls_d = nc.dram_tensor("smalls", [C, 2], f32, kind="ExternalInput")
    out_d = nc.dram_tensor("out", [C, N], f16, kind="ExternalOutput")

    DVE_MULT = float(A5 * SCALE)

    with tile.TileContext(nc) as tc:
        with tc.tile_pool(name="big", bufs=1) as big, \
             tc.tile_pool(name="small", bufs=1) as small, \
             tc.tile_pool(name="es_pool", bufs=8) as es_pool, \
             tc.tile_pool(name="work", bufs=2) as work, \
             tc.tile_pool(name="ps_sc", bufs=4, space="PSUM") as ps_sc, \
             tc.tile_pool(name="ps_pv", bufs=1, space="PSUM") as ps_pv, \
             tc.tile_pool(name="ps_rs", bufs=1, space="PSUM") as ps_rs:

            # x16 pieces: piece p covers n-cols [p*NB,(p+1)*NB) = key chunks
            # 8p..8p+7 (QK stationaries) + block p's residual + qproj moving.
            x16_t = [big.tile([C, NB], f16, tag=f"x16_{b}", name=f"x16_{b}")
                     for b in range(4)]
            q16_t = [big.tile([C, NB], f16, tag=f"q16_{b}", name=f"q16_{b}")
                     for b in range(4)]
            # v8 group g: v chunks 4g..4g+3 in [m, chunk, c] layout (fp8)
            v8_t = [big.tile([C, 4, C], f8e4, tag=f"v8_{g}", name=f"v8_{g}")
                    for g in range(8)]

            smalls = small.tile([C, 2], f32, tag="smalls")
            w16 = small.tile([C, 2 * C], f16, tag="w16")
            ones8 = small.tile([C, 2, C], f8e4, tag="ones8")
            ones16 = small.tile([C, C], f16, tag="ones16")
            wqwt16 = w16[:, 0:C]
            wvt16 = w16[:, C:2 * C]
            bqw = smalls[:, 0:1]
            bv16 = smalls[:, 1:2]

            nc.scalar.dma_start(w16[:], w16_d[:])
            nc.scalar.dma_start(smalls[:], smalls_d[:])
            # piece 0 gates qproj(0) -> first matmul: split it across the
            # sync + (idle) swdge queues so it lands ~1us earlier
            nc.sync.dma_start(x16_t[0][:, 0:512], x16_d[:, 0:512])
            nc.gpsimd.dma_start(x16_t[0][:, 512:NB], x16_d[:, 512:NB])
            for p in range(1, 4):
                eng = nc.sync if p % 2 == 0 else nc.scalar
                eng.dma_start(x16_t[p][:], x16_d[:, p * NB:(p + 1) * NB])
            nc.vector.memset(ones8[:], 1.0)
            nc.vector.memset(ones16[:], 1.0)

            def x16_chunk(j):
                p, r = divmod(j * C, NB)
                return x16_t[p][:, r:r + C]

            def qproj(b, on_dve):
                """q16 for block b: 2 fp16 matmuls + bias-adds split across
                DVE/ACT so both halves finish in parallel (gates first QK)."""
                for h in range(2):
                    qp = ps_sc.tile([C, 512], f32, tag="sc", name=f"qp{b}_{h}")
                    nc.tensor.matmul(qp[:], wqwt16,
                                     x16_t[b][:, h * 512:(h + 1) * 512],
                                     start=True, stop=True)
                    dst = q16_t[b][:, h * 512:(h + 1) * 512]
                    if (h == 0) == on_dve:
                        nc.vector.tensor_scalar(dst, qp[:], bqw, None, op0=ADD)
                    else:
                        nc.scalar.add(dst, qp[:], bqw)

            def vproj(g):
                """v chunks 4g..4g+3 (fp16 matmuls) -> v8 group g (fp8)."""
                vp = ps_sc.tile([C, 512], f32, tag="sc", name=f"vp{g}")
                for t in range(4):
                    nc.tensor.matmul(vp[:, t * C:(t + 1) * C],
                                     x16_chunk(4 * g + t), wvt16,
                                     start=True, stop=True)
                if g % 2 == 0:
                    nc.vector.tensor_scalar(v8_t[g][:], vp[:], bv16, None,
                                            op0=ADD)
                else:
                    nc.scalar.add(v8_t[g][:], vp[:], bv16)

            # exp engine per half-op: ACT exact exp vs DVE Schraudolph.
            ecnt = [0]

            def emit_exp(es_t, u, h, sc):
                dst = es_t[:, u, h * 512:(h + 1) * 512]
                on_act = (ecnt[0] * 5) % 9 < 5
                ecnt[0] += 1
                if on_act:
                    nc.scalar.activation(dst, sc[:], EXP, bias=0.0,
                                         scale=float(SCALE))
                else:
                    nc.vector.tensor_scalar(dst.bitcast(u8), sc[:],
                                            DVE_MULT, float(B5),
                                            op0=MUL, op1=ADD)

            def tail(b, pv, rs):
                n0 = b * NB
                last = b == NBLK - 1
                rb = work.tile([C, NB], f32, tag="rb", name=f"rb{b}")
                ep = work.tile([C, NB], f32, tag="ep", name=f"ep{b}")
                ost = work.tile([C, NB], f16, tag="ost", name=f"ost{b}")
                if last:
                    # exposed tail: quarter-grain DVE chain so each output
                    # quarter DMAs out while the next quarter computes
                    for q in range(4):
                        qsl = slice(q * 256, (q + 1) * 256)
                        nc.vector.reciprocal_approx_fast(out=rb[:, qsl],
                                                         in_=rs[:, qsl])
                        nc.vector.tensor_tensor(ep[:, qsl], pv[:, qsl],
                                                rb[:, qsl], op=MUL)
                        nc.vector.tensor_tensor(ost[:, qsl], ep[:, qsl],
                                                x16_t[b][:, qsl], op=ADD)
                        eng = nc.sync if q % 2 == 0 else nc.scalar
                        eng.dma_start(out_d[:, n0 + qsl.start:n0 + qsl.stop],
                                      ost[:, qsl])
                    return
                for h in range(2):
                    hsl = slice(h * 512, (h + 1) * 512)
                    nc.vector.reciprocal_approx_fast(out=rb[:, hsl],
                                                     in_=rs[:, hsl])
                    nc.vector.tensor_tensor(ep[:, hsl], pv[:, hsl], rb[:, hsl],
                                            op=MUL)
                    nc.gpsimd.tensor_tensor(ost[:, hsl], ep[:, hsl],
                                            x16_t[b][:, hsl], op=ADD)
                    eng = nc.sync if h == 0 else nc.scalar
                    eng.dma_start(out_d[:, n0 + hsl.start:n0 + hsl.stop],
                                  ost[:, hsl])

            qproj(0, on_dve=True)

            for b in range(NBLK):
                pv = ps_pv.tile([C, NB], f32, tag="pv", name=f"pv{b}")
                rs = ps_rs.tile([C, NB], f32, tag="rs", name=f"rs{b}")

                def flush_pv(pend):
                    # both PV halves share one v8-pair LDWEIGHTS, both RS
                    # halves share the ones8 load: 2 weight loads per pair
                    # instead of 4 (each reload stalls the PE ~160ns).
                    jp, et = pend
                    for h in range(2):
                        hsl = slice(h * 512, (h + 1) * 512)
                        nc.tensor.matmul(pv[:, hsl],
                                         v8_t[jp // 2][:, (jp % 2) * 2:(jp % 2) * 2 + 2, :],
                                         et[:, :, hsl],
                                         start=(jp == 0), stop=(jp == NPAIR - 1),
                                         perf_mode=DR)
                    for h in range(2):
                        hsl = slice(h * 512, (h + 1) * 512)
                        nc.tensor.matmul(rs[:, hsl], ones8[:],
                                         et[:, :, hsl],
                                         start=(jp == 0), stop=(jp == NPAIR - 1),
                                         perf_mode=DR)

                es_t = None
                pendq = []   # PV/RS deferred and flushed TWO pairs at a
                # time: fewer fp16<->fp8 mode transitions in the PE stream
                # (each first-matmul-after-transition costs ~150ns), and the
                # in-order PE always has fresh QK work before exp-gated reads
                for j in range(MCH):
                    if b == 0 and j % 4 == 0:
                        vproj(j // 4)
                    if j % 2 == 0:
                        es_t = es_pool.tile([C, 2, NB], f8e5, tag="es",
                                            name=f"es{b}_{j // 2}")
                    for h in range(2):
                        sc = ps_sc.tile([C, 512], f32, tag="sc",
                                        name=f"sc{b}_{j}_{h}")
                        nc.tensor.matmul(
                            sc[:], x16_chunk(j),
                            q16_t[b][:, h * 512:(h + 1) * 512],
                            start=True, stop=True)
                        emit_exp(es_t, j % 2, h, sc)
                    if j % 2 == 1:
                        pendq.append((j // 2, es_t))
                        if len(pendq) == 2:
                            if b == NBLK - 1 and j == MCH - 1:
                                # final flush: all RS first so the (RS-only)
                                # reciprocal chain overlaps the last PVs
                                for jp, et in pendq:
                                    for h in range(2):
                                        hsl = slice(h * 512, (h + 1) * 512)
                                        nc.tensor.matmul(
                                            rs[:, hsl], ones8[:], et[:, :, hsl],
                                            start=False, stop=(jp == NPAIR - 1),
                                            perf_mode=DR)
                                for jp, et in pendq:
                                    for h in range(2):
                                        hsl = slice(h * 512, (h + 1) * 512)
                                        nc.tensor.matmul(
                                            pv[:, hsl],
                                            v8_t[jp // 2][:, (jp % 2) * 2:(jp % 2) * 2 + 2, :],
                                            et[:, :, hsl],
                                            start=False, stop=(jp == NPAIR - 1),
                                            perf_mode=DR)
                            else:
                                for pend in pendq:
                                    flush_pv(pend)
                            pendq = []
                    if j == 19 and b < NBLK - 1:
                        qproj(b + 1, on_dve=(b % 2 == 0))
                for pend in pendq:
                    flush_pv(pend)
                tail(b, pv, rs)

    nc.finalize()
    return nc


def _get_nc():
    if "nc" not in _cache:
        _cache["nc"] = _build_nc()
    return _cache["nc"]


def make_in_maps(x, Wq, bq, Wk, bk, Wv, bv):
    x = np.asarray(x, dtype=np.float32)
    B = x.shape[0]
    wks = np.asarray(Wk, np.float32).sum(axis=0)            # [C]
    Wqw = np.asarray(Wq, np.float32) * wks[:, None]
    bqw = np.asarray(bq, np.float32) * wks
    w16 = np.ascontiguousarray(np.concatenate(
        [Wqw.T.astype(np.float16), np.asarray(Wv, np.float32).T.astype(np.float16)],
        axis=1))
    smalls = np.ascontiguousarray(
        np.stack([bqw, np.asarray(bv, np.float32)], axis=1).astype(np.float32))

    in_maps = []
    for i in range(B):
        xf = np.ascontiguousarray(x[i].reshape(C, N))
        in_maps.append({
            "x16": xf.astype(np.float16),
            "w16": w16, "smalls": smalls,
        })
    return in_maps


def kernel(x, Wq, bq, Wk, bk, Wv, bv, _trace=False, _tmpdir=None):
    from concourse.bass_utils import run_bass_kernel_spmd

    x = np.asarray(x, dtype=np.float32)
    B, c, H, W = x.shape
    assert (c, H * W) == (C, N), (c, H, W)
    in_maps = make_in_maps(x, Wq, bq, Wk, bk, Wv, bv)
    nc = _get_nc()
    res = run_bass_kernel_spmd(nc, in_maps, core_ids=list(range(B)),
                               trace=_trace, tmpdir=_tmpdir)
    out = np.stack([
        np.asarray(res.results[i]["out"]).astype(np.float32).reshape(C, H, W)
        for i in range(B)
    ])
    if _trace:
        _cache["last_result"] = res
    return out
